# revision 4
# baseline (speedup 1.0000x reference)
"""Trainium2 Bass kernel for a 2-layer leaky-integrate-and-fire SNN.

Model (per timestep t, snnTorch Leaky with reset-by-subtraction):
    cur1 = x_t @ w1.T + b1
    mem1 = beta*mem1_prev + cur1 - (mem1_prev > 1)          # threshold 1.0
    spk1 = (mem1 > 1)
    cur2 = spk1 @ w2.T + b2
    mem2 = beta*mem2_prev + cur2 - (mem2_prev > 1)
    spk2 = (mem2 > 1)
Outputs: spk2 (B,T,O) and mem2 (B,T,O).

Strategy (data-parallel over batch, 16 rows per core):
  * cur1 for ALL timesteps is a feed-forward GEMM (the recurrence is only
    elementwise).  It runs in fp16 with an error-compensated 3-term split
    x@w = xh@wh + xh@wl + xl@wh, xh = fp16(x), xl = fp16(x - xh): the LIF
    dynamics are chaotic (1e-4 cur1 noise => ~3k spike flips), so the GEMM
    must be fp32-faithful; fp16 3-term at 1 col/cycle is the cheapest way
    (fp32 matmul is 4 cyc/row, fp32r is tf32-precision).
  * Both scans run FUSED in the same DVE ops: the scan state is a
    (128, 144) tile = 8 h-chunks x 16 batch for layer 1 plus 16 extra
    columns holding layer 2 (partitions 0..9 = O), with layer 2 lagged one
    32-step block behind layer 1 (gemm2 of block s fills the layer-2
    columns of block s+1's tile).  With scaled state M = beta*mem:
        A:  M_t = (V_{t-1} * -beta) + beta*cur_t   (stt, in-place)
        B:  V_t = (M_t > beta) - M_t               (stt)
    This removes the 400 tiny (10,16) scan2 ops (~60us of DVE) for +16
    columns on the 400 scan1 ops.
  * Spikes (0/1, exact in fp16) are extracted blockwise on the DVE, one
    tensor_scalar per h-chunk, into fp16 tiles.
  * Layer-2: per h-chunk the fp16 stationary packs w2h at cols 0..9 and
    w2l at cols 32..41 (PSUM reads need 32-aligned starts) -> 8 matmuls
    per block; strips PSUM[0:10] + PSUM[32:42] combine at eviction,
    written into the NEXT block's tile layer-2 columns (and a drain
    buffer for the last 32 steps, which have no following block).
  * spk2 is computed on the host as (M2 > beta) -- bit-identical to the
    device compare -- so only M2 (= beta*mem2) is DMA'd out; the 1/beta
    un-scale also happens on host.
  * Output DMAs are issued from the Scalar queue so they never block
    input x DMA issue in the in-order Sync queue.

Tile layout: C tile (128, 32, 9, 16): partition p, local time t, chunk c
(c<8: h = c*128 + p; c=8: layer-2, partitions 0..9 = O), batch b.  Scan
slices C[:, t] are contiguous (128, 144).
"""

import numpy as np

BETA = 0.95
B, T, I, H, O = 128, 200, 784, 1024, 10
NCORES = 8
BL = B // NCORES          # 16 batch rows per core
TB = T * BL               # 3200 (t-major, b-minor columns)
HC = H // 128             # 8 h-chunks
LC = HC + 1               # chunks incl the layer-2 column group
TBLK = 32                 # timesteps per block
CHUNK = TBLK * BL         # 512 columns per block
NT = (32, 32, 32, 32, 32, 32, 8)     # timesteps per block
W2W = 32 + O              # w2h at cols 0..9, w2l at cols 32..41
NSUB = len(NT)
DRAIN = TBLK              # layer-2 steps drained after the last block

_nc_cache = None


def _build():
    import concourse.bacc as bacc
    import concourse.mybir as mybir
    from concourse.masks import make_identity as _make_identity
    from concourse.tile import TileContext

    Alu = mybir.AluOpType
    Act = mybir.ActivationFunctionType
    f32 = mybir.dt.float32
    f16 = mybir.dt.float16

    nc = bacc.Bacc("TRN2", target_bir_lowering=False, debug=False)

    KF = 6                # full 128-row contraction chunks (rows 0..767)
    KT = 48               # packed tail: [xh_t; xh_t; xl_t] x [w1h_t; w1l_t; w1h_t]
    xh_d = nc.dram_tensor("xh", (KF * 128, TB), f16, kind="ExternalInput")
    xl_d = nc.dram_tensor("xl", (KF * 128, TB), f16, kind="ExternalInput")
    xt_d = nc.dram_tensor("xt", (KT, TB), f16, kind="ExternalInput")
    w1h_d = nc.dram_tensor("w1h", (KF * 128, H), f16, kind="ExternalInput")
    w1l_d = nc.dram_tensor("w1l", (KF * 128, H), f16, kind="ExternalInput")
    w1t_d = nc.dram_tensor("w1t", (KT, H), f16, kind="ExternalInput")
    b1c = nc.dram_tensor("b1c", (128, HC), f32, kind="ExternalInput")
    w2p_d = nc.dram_tensor("w2p", (128, HC * W2W), f16, kind="ExternalInput")
    b2c = nc.dram_tensor("b2c", (O, 1), f32, kind="ExternalInput")
    M2 = nc.dram_tensor("M2", (O, TB), f32, kind="ExternalOutput")

    # block column starts
    blocks = []
    c0 = 0
    for n in NT:
        blocks.append((c0 * BL, n * BL))
        c0 += n
    assert c0 == T

    with TileContext(nc) as tc:
        with (
            tc.tile_pool(name="const", bufs=1) as cpool,
            tc.tile_pool(name="c1b", bufs=3) as c1pool,
            tc.tile_pool(name="xt", bufs=2) as xpool,
            tc.tile_pool(name="mv", bufs=2) as mvpool,
            tc.tile_pool(name="v2", bufs=2) as v2pool,
            tc.tile_pool(name="m2b", bufs=2) as m2pool,
            tc.tile_pool(name="ps1", bufs=4, space="PSUM") as ps1,
            tc.tile_pool(name="ps2", bufs=2, space="PSUM") as ps2,
        ):
            w1h_sb = cpool.tile([128, KF, H], f16)
            w1l_sb = cpool.tile([128, KF, H], f16)
            NP0 = CHUNK
            xh0 = xpool.tile([128, KF, CHUNK], f16, tag="xh", name="xh0")
            xl0 = xpool.tile([128, KF, CHUNK], f16, tag="xl", name="xl0")
            xt0 = xpool.tile([KT, CHUNK], f16, tag="xt", name="xt0")
            for k in range(KF):
                nc.sync.dma_start(
                    out=w1h_sb[:, k], in_=w1h_d[k * 128:(k + 1) * 128]
                )
                nc.sync.dma_start(
                    out=xh0[:, k, :NP0], in_=xh_d[k * 128:(k + 1) * 128, 0:NP0]
                )
                nc.sync.dma_start(
                    out=w1l_sb[:, k], in_=w1l_d[k * 128:(k + 1) * 128]
                )
                nc.sync.dma_start(
                    out=xl0[:, k, :NP0], in_=xl_d[k * 128:(k + 1) * 128, 0:NP0]
                )
            nc.sync.dma_start(out=xt0[:, :NP0], in_=xt_d[:, 0:NP0])
            w1t_sb = cpool.tile([KT, H], f16)
            nc.sync.dma_start(out=w1t_sb[:], in_=w1t_d[:])
            b1_sb = cpool.tile([128, HC], f32)
            nc.sync.dma_start(out=b1_sb[:], in_=b1c[:])
            w2p_sb = cpool.tile([128, HC, W2W], f16)
            b2_sb = cpool.tile([O, 1], f32)

            ident = cpool.tile([128, 128], f32)
            _make_identity(nc, ident[:])
            ttmp = cpool.tile([128, H], f32)
            dbuf = cpool.tile([O, DRAIN * BL], f32)    # layer-2 drain: steps 168..199

            v1 = mvpool.tile([128, LC * BL], f32, tag="v1")
            nc.vector.memset(v1[:], 0.0)

            c1_tiles = {}
            spk_tiles = {}

            def gemm1(p, ms):
                # one block's gemm1 for m-blocks in ms; x DMA + tile alloc
                # happen when ms[0] == 0
                c0, n = blocks[p]
                if ms[0] == 0:
                    if p == 0:
                        xh, xl, xt = xh0, xl0, xt0
                    else:
                        xh = xpool.tile([128, KF, CHUNK], f16, tag="xh")
                        xl = xpool.tile([128, KF, CHUNK], f16, tag="xl")
                        xt = xpool.tile([KT, CHUNK], f16, tag="xt")
                        for k in range(KF):
                            nc.sync.dma_start(
                                out=xh[:, k, :n],
                                in_=xh_d[k * 128:(k + 1) * 128, c0:c0 + n],
                            )
                            nc.sync.dma_start(
                                out=xl[:, k, :n],
                                in_=xl_d[k * 128:(k + 1) * 128, c0:c0 + n],
                            )
                        nc.sync.dma_start(out=xt[:, :n], in_=xt_d[:, c0:c0 + n])
                    gemm1.x = (xh, xl, xt)
                    c1_tiles[p] = c1pool.tile(
                        [128, TBLK, LC, BL], f32, tag="c1", name="c1"
                    )
                    spk_tiles[p] = c1pool.tile(
                        [128, HC, TBLK, BL], f16, tag="spk", name="spk"
                    )
                    if p == 0:
                        nc.vector.memset(c1_tiles[0][:, :, HC, :], 0.0)
                xh, xl, xt = gemm1.x

                def evict1(p1, m):
                    nt = n // BL
                    p1v = p1[:, :n].rearrange("p (t b) -> p t b", b=BL)
                    nc.scalar.activation(
                        out=c1_tiles[p][:, :nt, m, :],
                        in_=p1v[:, :nt, :],
                        func=Act.Identity,
                        bias=b1_sb[:, m:m + 1],
                        scale=1.0,
                    )

                def m_block(m):
                    p1 = ps1.tile([128, CHUNK], f32, tag="p1")
                    i = 0
                    for k in range(KF):
                        for (wt, xs_) in (
                            (w1h_sb, xh), (w1l_sb, xh), (w1h_sb, xl),
                        ):
                            nc.tensor.matmul(
                                p1[:, :n],
                                lhsT=wt[:, k, m * 128:(m + 1) * 128],
                                rhs=xs_[:, k, :n],
                                start=(i == 0),
                                stop=False,
                            )
                            i += 1
                    nc.tensor.matmul(
                        p1[:, :n],
                        lhsT=w1t_sb[:, m * 128:(m + 1) * 128],
                        rhs=xt[:, :n],
                        start=False,
                        stop=True,
                    )
                    evict1(p1, m)

                if n > 128:
                    if p == 0 and ms[0] == 0:
                        # Launch transient: run m=0,1 together k-outer (2x
                        # work per arriving x chunk) so the PE stays busy
                        # from chunk 0 and the HAM clock ramps early.
                        pA = ps1.tile([128, CHUNK], f32, tag="p1", name="pA")
                        pB = ps1.tile([128, CHUNK], f32, tag="p1", name="pB")
                        for k in range(KF):
                            for mi, pp in ((0, pA), (1, pB)):
                                for ti, (wt, xs_) in enumerate((
                                    (w1h_sb, xh), (w1l_sb, xh), (w1h_sb, xl),
                                )):
                                    nc.tensor.matmul(
                                        pp[:, :n],
                                        lhsT=wt[:, k, mi * 128:(mi + 1) * 128],
                                        rhs=xs_[:, k, :n],
                                        start=(k == 0 and ti == 0),
                                        stop=False,
                                    )
                        for mi, pp in ((0, pA), (1, pB)):
                            nc.tensor.matmul(
                                pp[:, :n],
                                lhsT=w1t_sb[:, mi * 128:(mi + 1) * 128],
                                rhs=xt[:, :n],
                                start=False,
                                stop=True,
                            )
                            evict1(pp, mi)
                        for m in ms:
                            if m >= 2:
                                m_block(m)
                    else:
                        for m in ms:
                            m_block(m)
                else:
                    # Short tail block (n=128): flip the orientation — x is
                    # stationary, w streams at N=512 — then transpose back.
                    # Runs whole-block on the first call; second call no-ops.
                    if ms[0] != 0:
                        return
                    for half in range(2):
                        p1 = ps1.tile([128, CHUNK], f32, tag="p1")
                        hs = slice(half * 512, (half + 1) * 512)
                        i = 0
                        for k in range(KF):
                            for (wt, xs_) in (
                                (w1h_sb, xh), (w1l_sb, xh), (w1h_sb, xl),
                            ):
                                nc.tensor.matmul(
                                    p1[:],
                                    lhsT=xs_[:, k, :n],
                                    rhs=wt[:, k, hs],
                                    start=(i == 0),
                                    stop=False,
                                )
                                i += 1
                        nc.tensor.matmul(
                            p1[:],
                            lhsT=xt[:, :n],
                            rhs=w1t_sb[:, hs],
                            start=False,
                            stop=True,
                        )
                        nc.scalar.activation(
                            out=ttmp[:, hs], in_=p1[:],
                            func=Act.Copy, bias=0.0, scale=1.0,
                        )
                    nt = n // BL
                    for m in range(HC):
                        pt = ps2.tile([128, 128], f32, tag="pt")
                        nc.tensor.transpose(
                            pt[:], ttmp[:, m * 128:(m + 1) * 128], ident[:]
                        )
                        ptv = pt.rearrange("p (t b) -> p t b", b=BL)
                        nc.scalar.activation(
                            out=c1_tiles[p][:, :nt, m, :],
                            in_=ptv[:, :nt, :],
                            func=Act.Identity,
                            bias=b1_sb[:, m:m + 1],
                            scale=1.0,
                        )

            def gemm2(s):
                # layer-2 GEMM for block s; strip-combined output lands in
                # the NEXT block's tile layer-2 columns (rows = layer-2
                # steps of block s), overflowing into dbuf when the next
                # block is shorter (s=5) or absent (s=6).
                c0, n = blocks[s]
                nt = n // BL
                spk = spk_tiles.pop(s)
                p2 = ps2.tile([W2W, CHUNK], f32, tag="p2")
                for c in range(HC):
                    nc.tensor.matmul(
                        p2[:, :n],
                        lhsT=w2p_sb[:, c, :],
                        rhs=spk[:, c, :nt, :],
                        start=(c == 0),
                        stop=(c == HC - 1),
                    )
                # strip combine: out = (psum[0:10] + b2) + psum[32:42]
                # into destination regions (C tile rows / drain buffer)
                dests = []
                nxt = nt if s + 1 >= NSUB else NT[s + 1]
                r = min(nt, nxt)
                if s + 1 < NSUB:
                    cn = c1_tiles[s + 1]
                    dests.append((0, r, cn[0:O, 0:r, HC, :]))
                else:
                    r = 0
                if r < nt:
                    d0 = (c0 // BL + r) - (T - DRAIN)   # col offset in dbuf
                    dests.append((r, nt, dbuf[:, d0 * BL:(d0 + nt - r) * BL]))
                for (t0, t1, dst) in dests:
                    nc.scalar.activation(
                        out=dst,
                        in_=p2[0:O, t0 * BL:t1 * BL],
                        func=Act.Identity,
                        bias=b2_sb[:, 0:1],
                        scale=1.0,
                    )
                    nc.vector.scalar_tensor_tensor(
                        out=dst, in0=p2[32:32 + O, t0 * BL:t1 * BL],
                        scalar=1.0, in1=dst,
                        op0=Alu.mult, op1=Alu.add,
                    )

            def emit_scans(s):
                # unified scan: one A/B stt pair per timestep over all 144
                # columns (layer 1 + lagged layer 2), then blockwise spike
                # extraction for layer 1.
                nonlocal v1
                c0, n = blocks[s]
                nt = n // BL
                c1 = c1_tiles[s]
                for j in range(nt):
                    csf = c1[:, j].rearrange("p c b -> p (c b)")
                    nc.vector.scalar_tensor_tensor(
                        out=csf, in0=v1[:], scalar=-BETA, in1=csf,
                        op0=Alu.mult, op1=Alu.add,
                    )
                    v1n = mvpool.tile([128, LC * BL], f32, tag="v1")
                    nc.vector.scalar_tensor_tensor(
                        out=v1n[:], in0=csf, scalar=BETA, in1=csf,
                        op0=Alu.is_gt, op1=Alu.subtract,
                    )
                    v1 = v1n
                spk = spk_tiles[s]
                for cix in range(HC):
                    nc.vector.tensor_scalar(
                        spk[:, cix, :nt, :],
                        c1[:, :nt, cix, :],
                        BETA, None, Alu.is_gt,
                    )

            def m2out(s):
                # M2 for layer-2 steps of block s-1, stored in C(s) rows:
                # pack to a contiguous buffer on ACT, DMA from the Scalar
                # queue so Sync's x DMAs are never blocked.
                if s == 0:
                    return
                pc0, pn = blocks[s - 1]
                nt = NT[s]
                m2b = m2pool.tile([O, TBLK * BL], f32, tag="m2b")
                nc.scalar.activation(
                    out=m2b[:, :nt * BL],
                    in_=c1_tiles[s][0:O, :nt, HC, :],
                    func=Act.Copy, bias=0.0, scale=1.0,
                )
                nc.scalar.dma_start(
                    out=M2[:, pc0:pc0 + nt * BL], in_=m2b[:, :nt * BL]
                )

            # Software pipeline (round r):
            #   PE:   gemm1(r) m0..m5, gemm2(r-1), gemm1(r) m6..m7
            #   DVE:  scan(r-1) + spike extraction (runs during round r)
            #   ACT:  evictions, strip-combines, M2 packs
            for bi in range(NSUB):
                gemm1(bi, range(0, 6))
                if bi == 0:
                    nc.sync.dma_start(out=w2p_sb[:], in_=w2p_d[:])
                    nc.sync.dma_start(out=b2_sb[:], in_=b2c[:])
                if bi > 0:
                    gemm2(bi - 1)
                    m2out(bi - 1)
                gemm1(bi, range(6, HC))
                emit_scans(bi)
            gemm2(NSUB - 1)
            m2out(NSUB - 1)
            # drain: last 32 layer-2 steps from dbuf
            vd = v1[0:O, HC * BL:HC * BL + BL]
            for j in range(DRAIN):
                ms = dbuf[:, j * BL:(j + 1) * BL]
                nc.vector.scalar_tensor_tensor(
                    out=ms, in0=vd, scalar=-BETA, in1=ms,
                    op0=Alu.mult, op1=Alu.add,
                )
                if j < DRAIN - 1:
                    v2n = v2pool.tile([O, BL], f32, tag="v2")
                    nc.vector.scalar_tensor_tensor(
                        out=v2n[:], in0=ms, scalar=BETA, in1=ms,
                        op0=Alu.is_gt, op1=Alu.subtract,
                    )
                    vd = v2n
            nc.scalar.dma_start(
                out=M2[:, (T - DRAIN) * BL:], in_=dbuf[:]
            )

    nc.compile()
    return nc


def _get_nc():
    global _nc_cache
    if _nc_cache is None:
        _nc_cache = _build()
    return _nc_cache


def _f16(a):
    return np.asarray(a, np.float16)


def _split16(a):
    hi = _f16(a)
    lo = _f16(np.asarray(a, np.float32) - hi.astype(np.float32))
    return hi, lo


def _prep_shared(w1, b1, w2, b2):
    w1s = (BETA * w1).T.astype(np.float32)        # (784, 1024)
    w1h_f, w1l_f = _split16(w1s)
    w1h = np.ascontiguousarray(w1h_f[:768])
    w1l = np.ascontiguousarray(w1l_f[:768])
    # packed 48-row tail: pairs (w1h,xh), (w1l,xh), (w1h,xl) in one matmul
    w1t = np.ascontiguousarray(
        np.concatenate([w1h_f[768:], w1l_f[768:], w1h_f[768:]], axis=0)
    )
    b1c = np.ascontiguousarray((BETA * b1).astype(np.float32).reshape(HC, 128).T)
    # GEMM2 consumes 0/1 spikes; stationary packs w2h at 0..9, w2l at 32..41
    w2s = (BETA * w2).T.astype(np.float32).reshape(HC, 128, O).transpose(1, 0, 2)
    w2h, w2l = _split16(np.ascontiguousarray(w2s))    # (128, HC, O) each
    w2p_a = np.zeros((128, HC, W2W), np.float16)
    w2p_a[:, :, :O] = w2h
    w2p_a[:, :, 32:32 + O] = w2l
    w2p = np.ascontiguousarray(w2p_a.reshape(128, HC * W2W))
    b2c = (BETA * b2).astype(np.float32).reshape(O, 1)
    return w1h, w1l, w1t, b1c, w2p, b2c


def _make_in_maps(x, w1, b1, w2, b2):
    w1h, w1l, w1t, b1c, w2p, b2c = _prep_shared(w1, b1, w2, b2)
    in_maps = []
    for c in range(NCORES):
        xs = x[c * BL:(c + 1) * BL]                     # (BL, T, I)
        xT = np.ascontiguousarray(
            xs.transpose(2, 1, 0).reshape(I, TB)        # col = t*BL + b
        )
        xh_f, xl_f = _split16(xT)
        xh = np.ascontiguousarray(xh_f[:768])
        xl = np.ascontiguousarray(xl_f[:768])
        xt = np.ascontiguousarray(
            np.concatenate([xh_f[768:], xh_f[768:], xl_f[768:]], axis=0)
        )
        in_maps.append({
            "xh": xh, "xl": xl, "xt": xt, "w1h": w1h, "w1l": w1l, "w1t": w1t,
            "b1c": b1c, "w2p": w2p, "b2c": b2c,
        })
    return in_maps


def kernel(x, w1, b1, w2, b2):
    from concourse.bass_utils import run_bass_kernel_spmd

    nc = _get_nc()
    in_maps = _make_in_maps(x, w1, b1, w2, b2)
    res = run_bass_kernel_spmd(nc, in_maps, core_ids=list(range(NCORES)))

    spk = np.empty((B, T, O), np.float32)
    mem = np.empty((B, T, O), np.float32)
    bf = np.float32(BETA)
    for c in range(NCORES):
        r = res.results[c]
        m2 = r["M2"].reshape(O, T, BL).transpose(2, 1, 0)
        spk[c * BL:(c + 1) * BL] = (m2 > bf).astype(np.float32)
        mem[c * BL:(c + 1) * BL] = m2 * np.float32(1.0 / BETA)
    return spk, mem


# revision 6
# speedup vs baseline: 1.0490x; 1.0490x over previous
"""Trainium2 Bass kernel for a 2-layer leaky-integrate-and-fire SNN.

Model (per timestep t, snnTorch Leaky with reset-by-subtraction):
    cur1 = x_t @ w1.T + b1
    mem1 = beta*mem1_prev + cur1 - (mem1_prev > 1)          # threshold 1.0
    spk1 = (mem1 > 1)
    cur2 = spk1 @ w2.T + b2
    mem2 = beta*mem2_prev + cur2 - (mem2_prev > 1)
    spk2 = (mem2 > 1)
Outputs: spk2 (B,T,O) and mem2 (B,T,O).

Strategy (data-parallel over batch, 16 rows per core):
  * cur1 for ALL timesteps is a feed-forward GEMM (the recurrence is only
    elementwise).  It runs in fp16 with an error-compensated 3-term split
    x@w = xh@wh + xh@wl + xl@wh, xh = fp16(x), xl = fp16(x - xh): the LIF
    dynamics are chaotic (1e-6 cur1 noise => ~10 spike flips, measured),
    so the GEMM must be fp32-faithful; fp16 3-term at 1 col/cycle is the
    cheapest route (fp32 matmul is 4 cyc/row, fp32r is tf32-precision).
    GEMM1 runs in 512-column physical blocks (the fp16 moving-operand ISA
    cap); the 128-col tail block flips orientation (x stationary).
  * Both scans run FUSED in the same DVE ops: the scan state is a
    (128, 144) tile = 8 h-chunks x 16 batch for layer 1 plus 16 extra
    columns holding layer 2 (partitions 0..9 = O), with layer 2 lagged 32
    steps behind layer 1 (gemm2 of scan-block s feeds the layer-2 columns
    of scan-block s+2).  With scaled state M = beta*mem:
        A:  M_t = (V_{t-1} * -beta) + beta*cur_t   (stt, in-place)
        B:  V_t = (M_t > beta) - M_t               (stt)
  * Scan blocks are 16 steps (half a physical gemm1 block) so the final
    uncovered scan after the PE finishes is small; the PE's gemm2 for
    block s runs one pblock-round after s's scan with 2 blocks of slack.
  * Spikes (0/1, exact in fp16) are extracted blockwise on the DVE into
    fp16 tiles; layer-2 stationary packs w2h at cols 0..9 and w2l at
    32..41 (PSUM reads need 32-aligned starts); strips PSUM[0:10] +
    PSUM[32:42] combine at eviction into the target scan tiles' layer-2
    columns.
  * The last 32 layer-2 steps have no following rows (the lag), so their
    inputs c2~ = beta*cur2 + beta*b2 are DMA'd out raw (D2) and the host
    finishes those 32 recurrence steps bit-exactly in fp32 (numpy IEEE
    ops match the DVE stt semantics: one rounding per ALU stage).
  * spk2 is computed on the host as (M2 > beta) -- bit-identical to the
    device compare; the 1/beta un-scale also happens on host.
  * Output DMAs are issued from the Scalar queue so they never block
    input x DMA issue in the in-order Sync queue.

Tile layout: C tile (128, 16, 9, 16): partition p, local time t, chunk c
(c<8: h = c*128 + p; c=8: layer-2, partitions 0..9 = O), batch b.  Scan
slices C[:, t] are contiguous (128, 144).
"""

import numpy as np

BETA = 0.95
B, T, I, H, O = 128, 200, 784, 1024, 10
NCORES = 8
BL = B // NCORES          # 16 batch rows per core
TB = T * BL               # 3200 (t-major, b-minor columns)
HC = H // 128             # 8 h-chunks
LC = HC + 1               # chunks incl the layer-2 column group
TBLK = 16                 # timesteps per scan block
NT = (16,) * 12 + (8,)    # timesteps per scan block
NSUB = len(NT)
PB = ((0, 512, 0), (512, 512, 2), (1024, 512, 4), (1536, 512, 6),
      (2048, 512, 8), (2560, 512, 10), (3072, 128, 12))  # (col0, ncols, sb0)
W2W = 32 + O              # w2h at cols 0..9, w2l at cols 32..41
LAG = 32                  # layer-2 step lag (2 scan blocks)
DRAIN = LAG               # layer-2 steps finished on the host

_nc_cache = None


def _sb_of(p):
    c0, n, s0 = PB[p]
    return list(range(s0, s0 + (2 if n == 512 else 1)))


def _build():
    import concourse.bacc as bacc
    import concourse.mybir as mybir
    from concourse.masks import make_identity as _make_identity
    from concourse.tile import TileContext

    Alu = mybir.AluOpType
    Act = mybir.ActivationFunctionType
    f32 = mybir.dt.float32
    f16 = mybir.dt.float16

    nc = bacc.Bacc("TRN2", target_bir_lowering=False, debug=False)

    KF = 6                # full 128-row contraction chunks (rows 0..767)
    KT = 48               # packed tail: [xh_t; xh_t; xl_t] x [w1h_t; w1l_t; w1h_t]
    xh_d = nc.dram_tensor("xh", (KF * 128, TB), f16, kind="ExternalInput")
    xl_d = nc.dram_tensor("xl", (KF * 128, TB), f16, kind="ExternalInput")
    xt_d = nc.dram_tensor("xt", (KT, TB), f16, kind="ExternalInput")
    w1h_d = nc.dram_tensor("w1h", (KF * 128, H), f16, kind="ExternalInput")
    w1l_d = nc.dram_tensor("w1l", (KF * 128, H), f16, kind="ExternalInput")
    w1t_d = nc.dram_tensor("w1t", (KT, H), f16, kind="ExternalInput")
    b1c = nc.dram_tensor("b1c", (128, HC), f32, kind="ExternalInput")
    w2p_d = nc.dram_tensor("w2p", (128, HC * W2W), f16, kind="ExternalInput")
    b2c = nc.dram_tensor("b2c", (O, 1), f32, kind="ExternalInput")
    M2 = nc.dram_tensor("M2", (O, (T - DRAIN) * BL), f32, kind="ExternalOutput")
    D2 = nc.dram_tensor("D2", (O, DRAIN * BL), f32, kind="ExternalOutput")

    # scan block starts (in timesteps)
    bstart = []
    c = 0
    for n in NT:
        bstart.append(c)
        c += n
    assert c == T

    with TileContext(nc) as tc:
        with (
            tc.tile_pool(name="const", bufs=1) as cpool,
            tc.tile_pool(name="c1b", bufs=6) as c1pool,
            tc.tile_pool(name="spkb", bufs=4) as spkpool,
            tc.tile_pool(name="xt", bufs=2) as xpool,
            tc.tile_pool(name="mv", bufs=2) as mvpool,
            tc.tile_pool(name="m2b", bufs=2) as m2pool,
            tc.tile_pool(name="ps1", bufs=4, space="PSUM") as ps1,
            tc.tile_pool(name="ps2", bufs=2, space="PSUM") as ps2,
        ):
            w1h_sb = cpool.tile([128, KF, H], f16)
            w1l_sb = cpool.tile([128, KF, H], f16)
            NP0 = PB[0][1]
            xh0 = xpool.tile([128, KF, 512], f16, tag="xh", name="xh0")
            xl0 = xpool.tile([128, KF, 512], f16, tag="xl", name="xl0")
            xt0 = xpool.tile([KT, 512], f16, tag="xt", name="xt0")
            for k in range(KF):
                nc.sync.dma_start(
                    out=w1h_sb[:, k], in_=w1h_d[k * 128:(k + 1) * 128]
                )
                nc.sync.dma_start(
                    out=xh0[:, k, :NP0], in_=xh_d[k * 128:(k + 1) * 128, 0:NP0]
                )
                nc.sync.dma_start(
                    out=w1l_sb[:, k], in_=w1l_d[k * 128:(k + 1) * 128]
                )
                nc.sync.dma_start(
                    out=xl0[:, k, :NP0], in_=xl_d[k * 128:(k + 1) * 128, 0:NP0]
                )
            nc.sync.dma_start(out=xt0[:, :NP0], in_=xt_d[:, 0:NP0])
            w1t_sb = cpool.tile([KT, H], f16)
            nc.sync.dma_start(out=w1t_sb[:], in_=w1t_d[:])
            b1_sb = cpool.tile([128, HC], f32)
            nc.sync.dma_start(out=b1_sb[:], in_=b1c[:])
            w2p_sb = cpool.tile([128, HC, W2W], f16)
            b2_sb = cpool.tile([O, 1], f32)

            ident = cpool.tile([128, 128], f32)
            _make_identity(nc, ident[:])
            ttmp = cpool.tile([128, H], f32)
            dbuf = cpool.tile([O, DRAIN * BL], f32)   # c2~ for host drain

            v1 = mvpool.tile([128, LC * BL], f32, tag="v1")
            nc.vector.memset(v1[:], 0.0)

            c1_tiles = {}
            spk_tiles = {}

            def gemm1(p, ms):
                # one pblock's gemm1 for m-blocks in ms; x DMA + tile alloc
                # happen when ms[0] == 0
                c0, n, s0 = PB[p]
                sbs = _sb_of(p)
                if ms[0] == 0:
                    if p == 0:
                        xh, xl, xt = xh0, xl0, xt0
                    else:
                        xh = xpool.tile([128, KF, 512], f16, tag="xh")
                        xl = xpool.tile([128, KF, 512], f16, tag="xl")
                        xt = xpool.tile([KT, 512], f16, tag="xt")
                        for k in range(KF):
                            nc.sync.dma_start(
                                out=xh[:, k, :n],
                                in_=xh_d[k * 128:(k + 1) * 128, c0:c0 + n],
                            )
                            nc.sync.dma_start(
                                out=xl[:, k, :n],
                                in_=xl_d[k * 128:(k + 1) * 128, c0:c0 + n],
                            )
                        nc.sync.dma_start(out=xt[:, :n], in_=xt_d[:, c0:c0 + n])
                    gemm1.x = (xh, xl, xt)
                    for s in sbs:
                        c1_tiles[s] = c1pool.tile(
                            [128, TBLK, LC, BL], f32, tag="c1", name="c1"
                        )
                        spk_tiles[s] = spkpool.tile(
                            [128, HC, TBLK, BL], f16, tag="spk", name="spk"
                        )
                        if s < 2:
                            nc.vector.memset(c1_tiles[s][:, :, HC, :], 0.0)
                xh, xl, xt = gemm1.x

                def evict1(p1, m):
                    for si, s in enumerate(sbs):
                        nt = NT[s]
                        p1v = p1[:, si * 256:si * 256 + nt * BL].rearrange(
                            "p (t b) -> p t b", b=BL
                        )
                        nc.scalar.activation(
                            out=c1_tiles[s][:, :nt, m, :],
                            in_=p1v,
                            func=Act.Identity,
                            bias=b1_sb[:, m:m + 1],
                            scale=1.0,
                        )

                def m_block(m):
                    p1 = ps1.tile([128, 512], f32, tag="p1")
                    i = 0
                    for k in range(KF):
                        for (wt, xs_) in (
                            (w1h_sb, xh), (w1l_sb, xh), (w1h_sb, xl),
                        ):
                            nc.tensor.matmul(
                                p1[:, :n],
                                lhsT=wt[:, k, m * 128:(m + 1) * 128],
                                rhs=xs_[:, k, :n],
                                start=(i == 0),
                                stop=False,
                            )
                            i += 1
                    nc.tensor.matmul(
                        p1[:, :n],
                        lhsT=w1t_sb[:, m * 128:(m + 1) * 128],
                        rhs=xt[:, :n],
                        start=False,
                        stop=True,
                    )
                    evict1(p1, m)

                if n > 128:
                    if p == 0 and ms[0] == 0:
                        # Launch transient: run m=0,1 together k-outer (2x
                        # work per arriving x chunk) so the PE stays busy
                        # from chunk 0 and the HAM clock ramps early.
                        pA = ps1.tile([128, 512], f32, tag="p1", name="pA")
                        pB = ps1.tile([128, 512], f32, tag="p1", name="pB")
                        for k in range(KF):
                            for mi, pp in ((0, pA), (1, pB)):
                                for ti, (wt, xs_) in enumerate((
                                    (w1h_sb, xh), (w1l_sb, xh), (w1h_sb, xl),
                                )):
                                    nc.tensor.matmul(
                                        pp[:, :n],
                                        lhsT=wt[:, k, mi * 128:(mi + 1) * 128],
                                        rhs=xs_[:, k, :n],
                                        start=(k == 0 and ti == 0),
                                        stop=False,
                                    )
                        for mi, pp in ((0, pA), (1, pB)):
                            nc.tensor.matmul(
                                pp[:, :n],
                                lhsT=w1t_sb[:, mi * 128:(mi + 1) * 128],
                                rhs=xt[:, :n],
                                start=False,
                                stop=True,
                            )
                            evict1(pp, mi)
                        for m in ms:
                            if m >= 2:
                                m_block(m)
                    else:
                        for m in ms:
                            m_block(m)
                else:
                    # Short tail pblock (n=128): flip the orientation — x is
                    # stationary, w streams at N=512 — then transpose back.
                    # Runs whole-block on the first call; second call no-ops.
                    if ms[0] != 0:
                        return
                    for half in range(2):
                        p1 = ps1.tile([128, 512], f32, tag="p1")
                        hs = slice(half * 512, (half + 1) * 512)
                        i = 0
                        for k in range(KF):
                            for (wt, xs_) in (
                                (w1h_sb, xh), (w1l_sb, xh), (w1h_sb, xl),
                            ):
                                nc.tensor.matmul(
                                    p1[:],
                                    lhsT=xs_[:, k, :n],
                                    rhs=wt[:, k, hs],
                                    start=(i == 0),
                                    stop=False,
                                )
                                i += 1
                        nc.tensor.matmul(
                            p1[:],
                            lhsT=xt[:, :n],
                            rhs=w1t_sb[:, hs],
                            start=False,
                            stop=True,
                        )
                        nc.scalar.activation(
                            out=ttmp[:, hs], in_=p1[:],
                            func=Act.Copy, bias=0.0, scale=1.0,
                        )
                    s = sbs[0]
                    nt = NT[s]
                    for m in range(HC):
                        pt = ps2.tile([128, 128], f32, tag="pt")
                        nc.tensor.transpose(
                            pt[:], ttmp[:, m * 128:(m + 1) * 128], ident[:]
                        )
                        ptv = pt.rearrange("p (t b) -> p t b", b=BL)
                        nc.scalar.activation(
                            out=c1_tiles[s][:, :nt, m, :],
                            in_=ptv[:, :nt, :],
                            func=Act.Identity,
                            bias=b1_sb[:, m:m + 1],
                            scale=1.0,
                        )

            def gemm2(s):
                # layer-2 GEMM for scan block s; strip-combined output lands
                # at layer-2 rows for steps (bstart[s]+LAG ..), i.e. tile
                # s+2's layer-2 columns, overflowing into dbuf for steps
                # beyond T-1 (host drain).
                nt = NT[s]
                spk = spk_tiles.pop(s)
                p2 = ps2.tile([W2W, TBLK * BL], f32, tag="p2")
                for c in range(HC):
                    nc.tensor.matmul(
                        p2[:, :nt * BL],
                        lhsT=w2p_sb[:, c, :],
                        rhs=spk[:, c, :nt, :],
                        start=(c == 0),
                        stop=(c == HC - 1),
                    )
                # destination rows: steps bstart[s]+32+j for j in 0..nt
                dests = []
                t0g = bstart[s] + LAG
                if s + 2 < NSUB:
                    r = min(nt, NT[s + 2])
                    dests.append((0, r, c1_tiles[s + 2][0:O, 0:r, HC, :]))
                else:
                    r = 0
                if r < nt:
                    d0 = (bstart[s] + r) - (T - DRAIN)
                    dests.append((r, nt, dbuf[:, d0 * BL:(d0 + nt - r) * BL]))
                for (t0, t1, dst) in dests:
                    nc.scalar.activation(
                        out=dst,
                        in_=p2[0:O, t0 * BL:t1 * BL],
                        func=Act.Identity,
                        bias=b2_sb[:, 0:1],
                        scale=1.0,
                    )
                    nc.vector.scalar_tensor_tensor(
                        out=dst, in0=p2[32:32 + O, t0 * BL:t1 * BL],
                        scalar=1.0, in1=dst,
                        op0=Alu.mult, op1=Alu.add,
                    )

            def emit_scans(s):
                # unified scan: one A/B stt pair per timestep over all 144
                # columns (layer 1 + lagged layer 2), then blockwise spike
                # extraction for layer 1.
                nonlocal v1
                nt = NT[s]
                c1 = c1_tiles[s]
                for j in range(nt):
                    csf = c1[:, j].rearrange("p c b -> p (c b)")
                    nc.vector.scalar_tensor_tensor(
                        out=csf, in0=v1[:], scalar=-BETA, in1=csf,
                        op0=Alu.mult, op1=Alu.add,
                    )
                    v1n = mvpool.tile([128, LC * BL], f32, tag="v1")
                    nc.vector.scalar_tensor_tensor(
                        out=v1n[:], in0=csf, scalar=BETA, in1=csf,
                        op0=Alu.is_gt, op1=Alu.subtract,
                    )
                    v1 = v1n
                spk = spk_tiles[s]
                for cix in range(HC):
                    nc.vector.tensor_scalar(
                        spk[:, cix, :nt, :],
                        c1[:, :nt, cix, :],
                        BETA, None, Alu.is_gt,
                    )

            def m2out(s):
                # M2 (= beta*mem2) for layer-2 steps bstart[s]-LAG..+NT[s],
                # stored in C(s) rows: pack on ACT, DMA from the Scalar
                # queue so Sync's x DMAs are never blocked.
                if s < 2:
                    return
                nt = NT[s]
                pc0 = (bstart[s] - LAG) * BL
                m2b = m2pool.tile([O, TBLK * BL], f32, tag="m2b")
                nc.scalar.activation(
                    out=m2b[:, :nt * BL],
                    in_=c1_tiles[s][0:O, :nt, HC, :],
                    func=Act.Copy, bias=0.0, scale=1.0,
                )
                nc.scalar.dma_start(
                    out=M2[:, pc0:pc0 + nt * BL], in_=m2b[:, :nt * BL]
                )

            # Software pipeline (round p over pblocks):
            #   PE:   gemm1(p) m0..m5, gemm2(prev pblock's blocks), m6..m7
            #   DVE:  scans of pblock p-1's blocks (run during round p)
            #   ACT:  evictions, strip-combines, M2 packs
            for p in range(len(PB)):
                gemm1(p, range(0, 6))
                if p == 0:
                    nc.sync.dma_start(out=w2p_sb[:], in_=w2p_d[:])
                    nc.sync.dma_start(out=b2_sb[:], in_=b2c[:])
                if p > 0:
                    prev = _sb_of(p - 1)
                    gemm2(prev[0])
                    m2out(prev[0])
                gemm1(p, range(6, HC))
                if p > 0 and len(_sb_of(p - 1)) > 1:
                    prev = _sb_of(p - 1)
                    gemm2(prev[1])
                    m2out(prev[1])
                for s in _sb_of(p):
                    emit_scans(s)
            # tail: gemm2 + M2 for the last pblock's block (s=12)
            gemm2(12)
            m2out(12)
            nc.scalar.dma_start(out=D2[:], in_=dbuf[:])

    nc.compile()
    return nc


def _get_nc():
    global _nc_cache
    if _nc_cache is None:
        _nc_cache = _build()
    return _nc_cache


def _f16(a):
    return np.asarray(a, np.float16)


def _split16(a):
    hi = _f16(a)
    lo = _f16(np.asarray(a, np.float32) - hi.astype(np.float32))
    return hi, lo


def _prep_shared(w1, b1, w2, b2):
    w1s = (BETA * w1).T.astype(np.float32)        # (784, 1024)
    w1h_f, w1l_f = _split16(w1s)
    w1h = np.ascontiguousarray(w1h_f[:768])
    w1l = np.ascontiguousarray(w1l_f[:768])
    # packed 48-row tail: pairs (w1h,xh), (w1l,xh), (w1h,xl) in one matmul
    w1t = np.ascontiguousarray(
        np.concatenate([w1h_f[768:], w1l_f[768:], w1h_f[768:]], axis=0)
    )
    b1c = np.ascontiguousarray((BETA * b1).astype(np.float32).reshape(HC, 128).T)
    # GEMM2 consumes 0/1 spikes; stationary packs w2h at 0..9, w2l at 32..41
    w2s = (BETA * w2).T.astype(np.float32).reshape(HC, 128, O).transpose(1, 0, 2)
    w2h, w2l = _split16(np.ascontiguousarray(w2s))    # (128, HC, O) each
    w2p_a = np.zeros((128, HC, W2W), np.float16)
    w2p_a[:, :, :O] = w2h
    w2p_a[:, :, 32:32 + O] = w2l
    w2p = np.ascontiguousarray(w2p_a.reshape(128, HC * W2W))
    b2c = (BETA * b2).astype(np.float32).reshape(O, 1)
    return w1h, w1l, w1t, b1c, w2p, b2c


def _make_in_maps(x, w1, b1, w2, b2):
    w1h, w1l, w1t, b1c, w2p, b2c = _prep_shared(w1, b1, w2, b2)
    in_maps = []
    for c in range(NCORES):
        xs = x[c * BL:(c + 1) * BL]                     # (BL, T, I)
        xT = np.ascontiguousarray(
            xs.transpose(2, 1, 0).reshape(I, TB)        # col = t*BL + b
        )
        xh_f, xl_f = _split16(xT)
        xh = np.ascontiguousarray(xh_f[:768])
        xl = np.ascontiguousarray(xl_f[:768])
        xt = np.ascontiguousarray(
            np.concatenate([xh_f[768:], xh_f[768:], xl_f[768:]], axis=0)
        )
        in_maps.append({
            "xh": xh, "xl": xl, "xt": xt, "w1h": w1h, "w1l": w1l, "w1t": w1t,
            "b1c": b1c, "w2p": w2p, "b2c": b2c,
        })
    return in_maps


def kernel(x, w1, b1, w2, b2):
    from concourse.bass_utils import run_bass_kernel_spmd

    nc = _get_nc()
    in_maps = _make_in_maps(x, w1, b1, w2, b2)
    res = run_bass_kernel_spmd(nc, in_maps, core_ids=list(range(NCORES)))

    spk = np.empty((B, T, O), np.float32)
    mem = np.empty((B, T, O), np.float32)
    bf = np.float32(BETA)
    nbf = np.float32(-BETA)
    for c in range(NCORES):
        r = res.results[c]
        m2a = r["M2"].reshape(O, T - DRAIN, BL)          # steps 0..167
        d2 = r["D2"].reshape(O, DRAIN, BL)               # c2~ for 168..199
        # host drain: finish the last DRAIN layer-2 steps bit-exactly
        # (each np.float32 op = one DVE ALU-stage rounding)
        Mlast = m2a[:, -1]                               # (O, BL), step 167
        V = ((Mlast > bf).astype(np.float32) - Mlast).astype(np.float32)
        md = np.empty((O, DRAIN, BL), np.float32)
        for j in range(DRAIN):
            Mj = ((V * nbf).astype(np.float32) + d2[:, j]).astype(np.float32)
            V = ((Mj > bf).astype(np.float32) - Mj).astype(np.float32)
            md[:, j] = Mj
        m2 = np.concatenate([m2a, md], axis=1).transpose(2, 1, 0)  # (BL,T,O)
        spk[c * BL:(c + 1) * BL] = (m2 > bf).astype(np.float32)
        mem[c * BL:(c + 1) * BL] = m2 * np.float32(1.0 / BETA)
    return spk, mem


# revision 13
# speedup vs baseline: 1.0801x; 1.0297x over previous
"""Trainium2 Bass kernel for a 2-layer leaky-integrate-and-fire SNN.

Model (per timestep t, snnTorch Leaky with reset-by-subtraction):
    cur1 = x_t @ w1.T + b1
    mem1 = beta*mem1_prev + cur1 - (mem1_prev > 1)          # threshold 1.0
    spk1 = (mem1 > 1)
    cur2 = spk1 @ w2.T + b2
    mem2 = beta*mem2_prev + cur2 - (mem2_prev > 1)
    spk2 = (mem2 > 1)
Outputs: spk2 (B,T,O) and mem2 (B,T,O).

Strategy (data-parallel over batch, 16 rows per core):
  * cur1 for ALL timesteps is a feed-forward GEMM (the recurrence is only
    elementwise).  It runs in fp16 with an error-compensated 3-term split
    x@w = xh@wh + xh@wl + xl@wh, xh = fp16(x), xl = fp16(x - xh): the LIF
    dynamics are chaotic (1e-6 cur1 noise => ~10 spike flips, measured),
    so the GEMM must be fp32-faithful; fp16 3-term at 1 col/cycle is the
    cheapest route (fp32 matmul is 4 cyc/row, fp32r is tf32-precision).
    GEMM1 runs in 512-column physical blocks (the fp16 moving-operand ISA
    cap); the 128-col tail block flips orientation (x stationary).
  * Both scans run FUSED in the same DVE ops: the scan state is a
    (128, 144) tile = 8 h-chunks x 16 batch for layer 1 plus 16 extra
    columns holding layer 2 (partitions 0..9 = O), with layer 2 lagged 32
    steps behind layer 1 (gemm2 of scan-block s feeds the layer-2 columns
    of scan-block s+2).  With scaled state M = beta*mem:
        A:  M_t = (V_{t-1} * -beta) + beta*cur_t   (stt, in-place)
        B:  V_t = (M_t > beta) - M_t               (stt)
  * Scan blocks are 16 steps (half a physical gemm1 block) so the final
    uncovered scan after the PE finishes is small; the PE's gemm2 for
    block s runs one pblock-round after s's scan with 2 blocks of slack.
  * Spikes (0/1, exact in fp16) are extracted blockwise on the DVE into
    fp16 tiles; layer-2 stationary packs w2h at cols 0..9 and w2l at
    32..41 (PSUM reads need 32-aligned starts); strips PSUM[0:10] +
    PSUM[32:42] combine at eviction into the target scan tiles' layer-2
    columns.
  * The last 32 layer-2 steps have no following rows (the lag), so their
    inputs c2~ = beta*cur2 + beta*b2 are DMA'd out raw (D2) and the host
    finishes those 32 recurrence steps bit-exactly in fp32 (numpy IEEE
    ops match the DVE stt semantics: one rounding per ALU stage).
  * spk2 is computed on the host as (M2 > beta) -- bit-identical to the
    device compare; the 1/beta un-scale also happens on host.
  * Output DMAs are issued from the Scalar queue so they never block
    input x DMA issue in the in-order Sync queue.

Tile layout: C tile (128, 16, 9, 16): partition p, local time t, chunk c
(c<8: h = c*128 + p; c=8: layer-2, partitions 0..9 = O), batch b.  Scan
slices C[:, t] are contiguous (128, 144).
"""

import numpy as np

BETA = 0.95
B, T, I, H, O = 128, 200, 784, 1024, 10
NCORES = 8
BL = B // NCORES          # 16 batch rows per core
TB = T * BL               # 3200 (t-major, b-minor columns)
HC = H // 128             # 8 h-chunks
LC = HC + 1               # chunks incl the layer-2 column group
TBLK = 16                 # timesteps per scan block
NT = (16,) * 12 + (8,)    # timesteps per scan block
NSUB = len(NT)
PB = ((0, 512, 0), (512, 512, 2), (1024, 512, 4), (1536, 512, 6),
      (2048, 512, 8), (2560, 512, 10), (3072, 128, 12))  # (col0, ncols, sb0)
W2W = 32 + O              # w2h at cols 0..9, w2l at cols 32..41
LAG = 32                  # layer-2 step lag (2 scan blocks)
DRAIN = LAG               # layer-2 steps finished on the host

_nc_cache = None


def _sb_of(p):
    c0, n, s0 = PB[p]
    return list(range(s0, s0 + (2 if n == 512 else 1)))


def _build():
    import concourse.bacc as bacc
    import concourse.mybir as mybir
    from concourse.tile import TileContext

    Alu = mybir.AluOpType
    Act = mybir.ActivationFunctionType
    f32 = mybir.dt.float32
    f16 = mybir.dt.float16

    nc = bacc.Bacc("TRN2", target_bir_lowering=False, debug=False)

    KF = 6                # full 128-row contraction chunks (rows 0..767)
    KT = 48               # packed tail: [xh_t; xh_t; xl_t] x [w1h_t; w1l_t; w1h_t]
    xh_d = nc.dram_tensor("xh", (KF * 128, TB), f16, kind="ExternalInput")
    xl_d = nc.dram_tensor("xl", (KF * 128, TB), f16, kind="ExternalInput")
    xt_d = nc.dram_tensor("xt", (KT, TB), f16, kind="ExternalInput")
    w1h_d = nc.dram_tensor("w1h", (KF * 128, H), f16, kind="ExternalInput")
    w1l_d = nc.dram_tensor("w1l", (KF * 128, H), f16, kind="ExternalInput")
    w1t_d = nc.dram_tensor("w1t", (KT, H), f16, kind="ExternalInput")
    b1c = nc.dram_tensor("b1c", (128, HC), f32, kind="ExternalInput")
    w2p_d = nc.dram_tensor("w2p", (128, HC * W2W), f16, kind="ExternalInput")
    b2c = nc.dram_tensor("b2c", (O, 1), f32, kind="ExternalInput")
    M2 = nc.dram_tensor("M2", (O, (T - DRAIN) * BL), f32, kind="ExternalOutput")
    D2 = nc.dram_tensor("D2", (O, DRAIN * BL), f32, kind="ExternalOutput")

    # scan block starts (in timesteps)
    bstart = []
    c = 0
    for n in NT:
        bstart.append(c)
        c += n
    assert c == T

    with TileContext(nc) as tc:
        with (
            tc.tile_pool(name="const", bufs=1) as cpool,
            tc.tile_pool(name="c1b", bufs=6) as c1pool,
            tc.tile_pool(name="spkb", bufs=4) as spkpool,
            tc.tile_pool(name="xt", bufs=2) as xpool,
            tc.tile_pool(name="mv", bufs=2) as mvpool,
            tc.tile_pool(name="m2b", bufs=2) as m2pool,
            tc.tile_pool(name="ps1", bufs=4, space="PSUM") as ps1,
            tc.tile_pool(name="ps2", bufs=2, space="PSUM") as ps2,
        ):
            w1h_sb = cpool.tile([128, KF, H], f16)
            w1l_sb = cpool.tile([128, KF, H], f16)
            NP0 = PB[0][1]
            xh0 = xpool.tile([128, KF, 512], f16, tag="xh", name="xh0")
            xl0 = xpool.tile([128, KF, 512], f16, tag="xl", name="xl0")
            xt0 = xpool.tile([KT, 512], f16, tag="xt", name="xt0")
            for k in range(KF):
                nc.sync.dma_start(
                    out=w1h_sb[:, k], in_=w1h_d[k * 128:(k + 1) * 128]
                )
                nc.sync.dma_start(
                    out=xh0[:, k, :NP0], in_=xh_d[k * 128:(k + 1) * 128, 0:NP0]
                )
                nc.sync.dma_start(
                    out=w1l_sb[:, k], in_=w1l_d[k * 128:(k + 1) * 128]
                )
                nc.sync.dma_start(
                    out=xl0[:, k, :NP0], in_=xl_d[k * 128:(k + 1) * 128, 0:NP0]
                )
            nc.sync.dma_start(out=xt0[:, :NP0], in_=xt_d[:, 0:NP0])
            w1t_sb = cpool.tile([KT, H], f16)
            nc.sync.dma_start(out=w1t_sb[:], in_=w1t_d[:])
            b1_sb = cpool.tile([128, HC], f32)
            nc.sync.dma_start(out=b1_sb[:], in_=b1c[:])
            w2p_sb = cpool.tile([128, HC, W2W], f16)
            b2_sb = cpool.tile([O, 1], f32)

            dbuf = cpool.tile([O, DRAIN * BL], f32)   # c2~ for host drain

            v1 = mvpool.tile([128, LC * BL], f32, tag="v1")
            nc.vector.memset(v1[:], 0.0)

            c1_tiles = {}
            spk_tiles = {}

            def gemm1(p, part):
                # one pblock's gemm1, in two emission parts so gemm2 of the
                # previous pblock slots between them on the PE queue.
                # Regular pblocks: part 0 = m0..m5, part 1 = m6..m7.
                # p=5 runs N=256 halves (part = half, all m) so C(10) evicts
                # at mid-round and its scan overlaps this pblock's PE round.
                c0, n, s0 = PB[p]
                sbs = _sb_of(p)
                if part == 0:
                    if p == 0:
                        xh, xl, xt = xh0, xl0, xt0
                    else:
                        xh = xpool.tile([128, KF, 512], f16, tag="xh")
                        xl = xpool.tile([128, KF, 512], f16, tag="xl")
                        xt = xpool.tile([KT, 512], f16, tag="xt")
                        for k in range(KF):
                            nc.sync.dma_start(
                                out=xh[:, k, :n],
                                in_=xh_d[k * 128:(k + 1) * 128, c0:c0 + n],
                            )
                            nc.sync.dma_start(
                                out=xl[:, k, :n],
                                in_=xl_d[k * 128:(k + 1) * 128, c0:c0 + n],
                            )
                        nc.sync.dma_start(out=xt[:, :n], in_=xt_d[:, c0:c0 + n])
                    gemm1.x = (xh, xl, xt)
                    for s in sbs:
                        c1_tiles[s] = c1pool.tile(
                            [128, TBLK, LC, BL], f32, tag="c1", name="c1"
                        )
                        spk_tiles[s] = spkpool.tile(
                            [128, HC, TBLK, BL], f16, tag="spk", name="spk"
                        )
                        if s < 2:
                            nc.vector.memset(c1_tiles[s][:, :, HC, :], 0.0)
                xh, xl, xt = gemm1.x

                def evict1(p1, m):
                    for si, s in enumerate(sbs):
                        nt = NT[s]
                        p1v = p1[:, si * 256:si * 256 + nt * BL].rearrange(
                            "p (t b) -> p t b", b=BL
                        )
                        nc.scalar.activation(
                            out=c1_tiles[s][:, :nt, m, :],
                            in_=p1v,
                            func=Act.Identity,
                            bias=b1_sb[:, m:m + 1],
                            scale=1.0,
                        )

                def m_block(m, coff=0, nn=None, sb=None):
                    nn = n if nn is None else nn
                    p1 = ps1.tile([128, 512], f32, tag="p1")
                    i = 0
                    for k in range(KF):
                        for (wt, xs_) in (
                            (w1h_sb, xh), (w1l_sb, xh), (w1h_sb, xl),
                        ):
                            nc.tensor.matmul(
                                p1[:, :nn],
                                lhsT=wt[:, k, m * 128:(m + 1) * 128],
                                rhs=xs_[:, k, coff:coff + nn],
                                start=(i == 0),
                                stop=False,
                            )
                            i += 1
                    nc.tensor.matmul(
                        p1[:, :nn],
                        lhsT=w1t_sb[:, m * 128:(m + 1) * 128],
                        rhs=xt[:, coff:coff + nn],
                        start=False,
                        stop=True,
                    )
                    if sb is None:
                        evict1(p1, m)
                    else:
                        nt = NT[sb]
                        p1v = p1[:, :nt * BL].rearrange(
                            "p (t b) -> p t b", b=BL
                        )
                        nc.scalar.activation(
                            out=c1_tiles[sb][:, :nt, m, :],
                            in_=p1v,
                            func=Act.Identity,
                            bias=b1_sb[:, m:m + 1],
                            scale=1.0,
                        )

                if p == 5:
                    # N=256 halves: all 8 m-blocks for scan block 10+part
                    for m in range(HC):
                        m_block(m, coff=part * 256, nn=256, sb=10 + part)
                elif n > 128:
                    ms = range(0, 6) if part == 0 else range(6, HC)
                    if p == 0 and part == 0:
                        # Launch transient: run m=0,1 together k-outer (2x
                        # work per arriving x chunk) so the PE stays busy
                        # from chunk 0 and the HAM clock ramps early.
                        pA = ps1.tile([128, 512], f32, tag="p1", name="pA")
                        pB = ps1.tile([128, 512], f32, tag="p1", name="pB")
                        for k in range(KF):
                            for mi, pp in ((0, pA), (1, pB)):
                                for ti, (wt, xs_) in enumerate((
                                    (w1h_sb, xh), (w1l_sb, xh), (w1h_sb, xl),
                                )):
                                    nc.tensor.matmul(
                                        pp[:, :n],
                                        lhsT=wt[:, k, mi * 128:(mi + 1) * 128],
                                        rhs=xs_[:, k, :n],
                                        start=(k == 0 and ti == 0),
                                        stop=False,
                                    )
                        for mi, pp in ((0, pA), (1, pB)):
                            nc.tensor.matmul(
                                pp[:, :n],
                                lhsT=w1t_sb[:, mi * 128:(mi + 1) * 128],
                                rhs=xt[:, :n],
                                start=False,
                                stop=True,
                            )
                            evict1(pp, mi)
                        for m in ms:
                            if m >= 2:
                                m_block(m)
                    else:
                        for m in ms:
                            m_block(m)
                else:
                    # Short tail pblock (n=128), straight N=128 matmuls:
                    # short streams, FWL-hidden weight loads.
                    ms = range(0, 6) if part == 0 else range(6, HC)
                    for m in ms:
                        m_block(m, nn=128, sb=sbs[0])

            def gemm2(s):
                # layer-2 GEMM for scan block s; strip-combined output lands
                # at layer-2 rows for steps (bstart[s]+LAG ..), i.e. tile
                # s+2's layer-2 columns, overflowing into dbuf for steps
                # beyond T-1 (host drain).
                nt = NT[s]
                spk = spk_tiles.pop(s)
                p2 = ps2.tile([W2W, TBLK * BL], f32, tag="p2")
                for c in range(HC):
                    nc.tensor.matmul(
                        p2[:, :nt * BL],
                        lhsT=w2p_sb[:, c, :],
                        rhs=spk[:, c, :nt, :],
                        start=(c == 0),
                        stop=(c == HC - 1),
                    )
                # destination rows: steps bstart[s]+32+j for j in 0..nt
                dests = []
                t0g = bstart[s] + LAG
                if s + 2 < NSUB:
                    r = min(nt, NT[s + 2])
                    dests.append((0, r, c1_tiles[s + 2][0:O, 0:r, HC, :]))
                else:
                    r = 0
                if r < nt:
                    d0 = (bstart[s] + r) - (T - DRAIN)
                    dests.append((r, nt, dbuf[:, d0 * BL:(d0 + nt - r) * BL]))
                for (t0, t1, dst) in dests:
                    nc.scalar.activation(
                        out=dst,
                        in_=p2[0:O, t0 * BL:t1 * BL],
                        func=Act.Identity,
                        bias=b2_sb[:, 0:1],
                        scale=1.0,
                    )
                    nc.vector.scalar_tensor_tensor(
                        out=dst, in0=p2[32:32 + O, t0 * BL:t1 * BL],
                        scalar=1.0, in1=dst,
                        op0=Alu.mult, op1=Alu.add,
                    )

            def emit_scans(s):
                # unified scan: one A/B stt pair per timestep over all 144
                # columns (layer 1 + lagged layer 2), then blockwise spike
                # extraction for layer 1.
                nonlocal v1
                nt = NT[s]
                c1 = c1_tiles[s]
                for j in range(nt):
                    csf = c1[:, j].rearrange("p c b -> p (c b)")
                    nc.vector.scalar_tensor_tensor(
                        out=csf, in0=v1[:], scalar=-BETA, in1=csf,
                        op0=Alu.mult, op1=Alu.add,
                    )
                    v1n = mvpool.tile([128, LC * BL], f32, tag="v1")
                    nc.vector.scalar_tensor_tensor(
                        out=v1n[:], in0=csf, scalar=BETA, in1=csf,
                        op0=Alu.is_gt, op1=Alu.subtract,
                    )
                    v1 = v1n
                spk = spk_tiles[s]
                for cix in range(HC):
                    nc.vector.tensor_scalar(
                        spk[:, cix, :nt, :],
                        c1[:, :nt, cix, :],
                        BETA, None, Alu.is_gt,
                    )

            def m2out(s):
                # M2 (= beta*mem2) for layer-2 steps bstart[s]-LAG..+NT[s],
                # stored in C(s) rows: pack on ACT, DMA from the Scalar
                # queue so Sync's x DMAs are never blocked.
                if s < 2:
                    return
                nt = NT[s]
                pc0 = (bstart[s] - LAG) * BL
                m2b = m2pool.tile([O, TBLK * BL], f32, tag="m2b")
                nc.scalar.activation(
                    out=m2b[:, :nt * BL],
                    in_=c1_tiles[s][0:O, :nt, HC, :],
                    func=Act.Copy, bias=0.0, scale=1.0,
                )
                nc.scalar.dma_start(
                    out=M2[:, pc0:pc0 + nt * BL], in_=m2b[:, :nt * BL]
                )

            # Software pipeline (round p over pblocks):
            #   PE:   gemm1(p) m0..m5, gemm2(prev pblock's blocks), m6..m7
            #   DVE:  scans of pblock p-1's blocks (run during round p)
            #   ACT:  evictions, strip-combines, M2 packs
            for p in range(len(PB)):
                gemm1(p, 0)
                if p == 0:
                    nc.sync.dma_start(out=w2p_sb[:], in_=w2p_d[:])
                    nc.sync.dma_start(out=b2_sb[:], in_=b2c[:])
                if p > 0:
                    prev = _sb_of(p - 1)
                    gemm2(prev[0])
                    m2out(prev[0])
                gemm1(p, 1)
                if p > 0 and len(_sb_of(p - 1)) > 1:
                    prev = _sb_of(p - 1)
                    gemm2(prev[1])
                    m2out(prev[1])
                for s in _sb_of(p):
                    emit_scans(s)
            # tail: gemm2 + M2 for the last pblock's block (s=12)
            gemm2(12)
            m2out(12)
            nc.scalar.dma_start(out=D2[:], in_=dbuf[:])

    nc.compile()
    return nc


def _get_nc():
    global _nc_cache
    if _nc_cache is None:
        _nc_cache = _build()
    return _nc_cache


def _f16(a):
    return np.asarray(a, np.float16)


def _split16(a):
    hi = _f16(a)
    lo = _f16(np.asarray(a, np.float32) - hi.astype(np.float32))
    return hi, lo


def _prep_shared(w1, b1, w2, b2):
    w1s = (BETA * w1).T.astype(np.float32)        # (784, 1024)
    w1h_f, w1l_f = _split16(w1s)
    w1h = np.ascontiguousarray(w1h_f[:768])
    w1l = np.ascontiguousarray(w1l_f[:768])
    # packed 48-row tail: pairs (w1h,xh), (w1l,xh), (w1h,xl) in one matmul
    w1t = np.ascontiguousarray(
        np.concatenate([w1h_f[768:], w1l_f[768:], w1h_f[768:]], axis=0)
    )
    b1c = np.ascontiguousarray((BETA * b1).astype(np.float32).reshape(HC, 128).T)
    # GEMM2 consumes 0/1 spikes; stationary packs w2h at 0..9, w2l at 32..41
    w2s = (BETA * w2).T.astype(np.float32).reshape(HC, 128, O).transpose(1, 0, 2)
    w2h, w2l = _split16(np.ascontiguousarray(w2s))    # (128, HC, O) each
    w2p_a = np.zeros((128, HC, W2W), np.float16)
    w2p_a[:, :, :O] = w2h
    w2p_a[:, :, 32:32 + O] = w2l
    w2p = np.ascontiguousarray(w2p_a.reshape(128, HC * W2W))
    b2c = (BETA * b2).astype(np.float32).reshape(O, 1)
    return w1h, w1l, w1t, b1c, w2p, b2c


def _make_in_maps(x, w1, b1, w2, b2):
    w1h, w1l, w1t, b1c, w2p, b2c = _prep_shared(w1, b1, w2, b2)
    in_maps = []
    for c in range(NCORES):
        xs = x[c * BL:(c + 1) * BL]                     # (BL, T, I)
        xT = np.ascontiguousarray(
            xs.transpose(2, 1, 0).reshape(I, TB)        # col = t*BL + b
        )
        xh_f, xl_f = _split16(xT)
        xh = np.ascontiguousarray(xh_f[:768])
        xl = np.ascontiguousarray(xl_f[:768])
        xt = np.ascontiguousarray(
            np.concatenate([xh_f[768:], xh_f[768:], xl_f[768:]], axis=0)
        )
        in_maps.append({
            "xh": xh, "xl": xl, "xt": xt, "w1h": w1h, "w1l": w1l, "w1t": w1t,
            "b1c": b1c, "w2p": w2p, "b2c": b2c,
        })
    return in_maps


def kernel(x, w1, b1, w2, b2):
    from concourse.bass_utils import run_bass_kernel_spmd

    nc = _get_nc()
    in_maps = _make_in_maps(x, w1, b1, w2, b2)
    res = run_bass_kernel_spmd(nc, in_maps, core_ids=list(range(NCORES)))

    spk = np.empty((B, T, O), np.float32)
    mem = np.empty((B, T, O), np.float32)
    bf = np.float32(BETA)
    nbf = np.float32(-BETA)
    for c in range(NCORES):
        r = res.results[c]
        m2a = r["M2"].reshape(O, T - DRAIN, BL)          # steps 0..167
        d2 = r["D2"].reshape(O, DRAIN, BL)               # c2~ for 168..199
        # host drain: finish the last DRAIN layer-2 steps bit-exactly
        # (each np.float32 op = one DVE ALU-stage rounding)
        Mlast = m2a[:, -1]                               # (O, BL), step 167
        V = ((Mlast > bf).astype(np.float32) - Mlast).astype(np.float32)
        md = np.empty((O, DRAIN, BL), np.float32)
        for j in range(DRAIN):
            Mj = ((V * nbf).astype(np.float32) + d2[:, j]).astype(np.float32)
            V = ((Mj > bf).astype(np.float32) - Mj).astype(np.float32)
            md[:, j] = Mj
        m2 = np.concatenate([m2a, md], axis=1).transpose(2, 1, 0)  # (BL,T,O)
        spk[c * BL:(c + 1) * BL] = (m2 > bf).astype(np.float32)
        mem[c * BL:(c + 1) * BL] = m2 * np.float32(1.0 / BETA)
    return spk, mem
